# revision 7
# baseline (speedup 1.0000x reference)
"""Trainium2 Bass kernel for nn_GCN_15590731285230.

Reference computation (per batch b of B=8, N=2048, D=128):
    R  = softmax(x_b @ wr @ x_b^T, axis=-1)          # [N, N]
    h1 = relu(R @ x_b @ w1 + x_b)                    # [N, D]
    h2 = relu(R @ h1 @ w2 + h1)                      # [N, D]
    out_b = h2[0, :]                                 # [D]

Key algorithmic facts exploited here:
  * Only row 0 of layer 2 is needed:  out_b = relu(R[0,:] @ h1 @ w2 + h1[0,:]),
    so the second N^2 D matmul collapses to a vector-matrix product.
  * The N x N score matrix never goes to HBM: scores are produced block-by-block
    in PSUM, softmaxed in place, and consumed by the h1 matmul (flash style).

Sharding: data-parallel over batch. Core i handles batch element i; weights are
replicated. Each core's program is identical (SPMD via run_bass_kernel_spmd).

Layout conventions inside a core (SBUF tiles are [partition, free]):
  xnat [128, 16, 128] : x rows, row (t*128+p) at [p, t, :]  (m/n on partitions)
  xT   [128, 2048]    : x^T, features on partitions
  yT   [128, 2048]    : (x @ wr)^T, features on partitions
  For each 128-row block nb of n:
    S    = yT[:, nb]^T @ xT          -> PSUM [128 n, 2048 m]   (scores)
    E    = exp(S - rowmax) / rowsum  -> SBUF (softmax weights, row-normalized)
    ZT   = sum_t  x_chunk_t^T-form matmuls with transposed E chunks
           -> PSUM [128 f, 128 n]    ((R @ x)^T for this block)
    h1T  = relu(w1^T-form matmul + xT block), transposed back into h1nat.
  Finally: v = h1^T r0 (r0 = R[0,:], taken from transposed E chunks of block 0),
  out = relu(v^T @ w2 + h1[0,:]).

`repeat` (timing only): emits the whole body `repeat` times straight-line, so
per-iteration device time can be extracted from wall-clock differences without
paying the ~0.5 s per-dispatch host overhead per sample.
"""

import sys

if "/opt/trn_rl_repo" not in sys.path:
    sys.path.insert(0, "/opt/trn_rl_repo")

from contextlib import ExitStack

import numpy as np

import concourse.bacc as bacc
import concourse.mybir as mybir
import concourse.tile as tile
from concourse.bass_utils import run_bass_kernel_spmd
from concourse.masks import make_identity

P = 128          # partitions / feature dim D
D = 128
B = 8
F32 = mybir.dt.float32
AF = mybir.ActivationFunctionType
AX = mybir.AxisListType


def build_kernel(n=2048, repeat=1):
    nb_cnt = n // P          # row blocks
    hw = n // 2              # S half width (PSUM bank budget)
    nc = bacc.Bacc()
    x_d = nc.dram_tensor("x", [n, D], F32, kind="ExternalInput")
    wr_d = nc.dram_tensor("wr", [D, D], F32, kind="ExternalInput")
    w1_d = nc.dram_tensor("w1", [D, D], F32, kind="ExternalInput")
    w2_d = nc.dram_tensor("w2", [D, D], F32, kind="ExternalInput")
    out_d = nc.dram_tensor("out", [1, D], F32, kind="ExternalOutput")

    with tile.TileContext(nc) as tc, ExitStack() as ctx:
        # PSUM budget (8 banks total): psS 2x[128,1024]=4, psT 2x[128,128]=2,
        # psZ 1, psH 1.
        singles = ctx.enter_context(tc.tile_pool(name="singles", bufs=1))
        psS = ctx.enter_context(tc.tile_pool(name="psS", bufs=2, space="PSUM"))
        psT = ctx.enter_context(tc.tile_pool(name="psT", bufs=2, space="PSUM"))
        psZ = ctx.enter_context(tc.tile_pool(name="psZ", bufs=1, space="PSUM"))
        psH = ctx.enter_context(tc.tile_pool(name="psH", bufs=1, space="PSUM"))
        epool = ctx.enter_context(tc.tile_pool(name="epool", bufs=2))
        etp = ctx.enter_context(tc.tile_pool(name="etp", bufs=3))
        ztp = ctx.enter_context(tc.tile_pool(name="ztp", bufs=2))
        stat = ctx.enter_context(tc.tile_pool(name="stat", bufs=6))
        h1tp = ctx.enter_context(tc.tile_pool(name="h1tp", bufs=2))

        for _rep in range(repeat):
            ident = singles.tile([P, P], F32, tag="ident")
            make_identity(nc, ident)
            wr_sb = singles.tile([P, P], F32, tag="wr_sb")
            w1_sb = singles.tile([P, P], F32, tag="w1_sb")
            w2_sb = singles.tile([P, P], F32, tag="w2_sb")
            nc.sync.dma_start(wr_sb, wr_d[:])
            nc.sync.dma_start(w1_sb, w1_d[:])
            nc.sync.dma_start(w2_sb, w2_d[:])

            # x natural: one contiguous 64KB DMA per 128-row chunk
            xnat = singles.tile([P, nb_cnt, P], F32, tag="xnat")
            x_blk = x_d[:].rearrange("(t p) f -> t p f", p=P)
            for t in range(nb_cnt):
                nc.sync.dma_start(xnat[:, t, :], x_blk[t])

            xT = singles.tile([P, n], F32, tag="xT")
            yT = singles.tile([P, n], F32, tag="yT")
            for t in range(nb_cnt):
                tp = psT.tile([P, P], F32, tag="tp")
                nc.tensor.transpose(tp, xnat[:, t, :], ident)
                nc.vector.tensor_copy(xT[:, t * P:(t + 1) * P], tp)
            # yT = (x @ wr)^T = wr^T-form matmul over xT
            for j in range(0, n, 512):
                w = min(512, n - j)
                yp = psS.tile([P, hw], F32, tag="s")
                nc.tensor.matmul(yp[:, :w], lhsT=wr_sb, rhs=xT[:, j:j + w],
                                 start=True, stop=True)
                nc.vector.tensor_copy(yT[:, j:j + w], yp[:, :w])

            # persistent outputs of the block loop
            h1nat = singles.tile([P, nb_cnt, P], F32, tag="h1nat")
            r0 = singles.tile([P, nb_cnt], F32, tag="r0")

            for nb in range(nb_cnt):
                ncol = nb * P
                # scores S[nb block, :] in two PSUM halves
                s0 = psS.tile([P, hw], F32, tag="s")
                s1 = psS.tile([P, hw], F32, tag="s")
                for h, sp in enumerate((s0, s1)):
                    for j in range(0, hw, 512):
                        w = min(512, hw - j)
                        nc.tensor.matmul(sp[:, j:j + w],
                                         lhsT=yT[:, ncol:ncol + P],
                                         rhs=xT[:, h * hw + j:h * hw + j + w],
                                         start=True, stop=True)
                # row softmax (stable), normalized in SBUF
                mx0 = stat.tile([P, 1], F32, tag="mx")
                mx1 = stat.tile([P, 1], F32, tag="mx")
                nc.vector.reduce_max(mx0, s0, axis=AX.X)
                nc.vector.reduce_max(mx1, s1, axis=AX.X)
                negmx = stat.tile([P, 1], F32, tag="mx")
                nc.vector.tensor_max(negmx, mx0, mx1)
                nc.vector.tensor_scalar_mul(negmx, negmx, -1.0)
                e = epool.tile([P, n], F32, tag="e")
                sum0 = stat.tile([P, 1], F32, tag="ss")
                sum1 = stat.tile([P, 1], F32, tag="ss")
                nc.scalar.activation(e[:, 0:hw], s0, AF.Exp, bias=negmx,
                                     accum_out=sum0)
                nc.scalar.activation(e[:, hw:n], s1, AF.Exp, bias=negmx,
                                     accum_out=sum1)
                rs = stat.tile([P, 1], F32, tag="ss")
                nc.vector.tensor_add(rs, sum0, sum1)
                nc.vector.reciprocal(rs, rs)
                nc.vector.tensor_scalar_mul(e, e, rs)

                # ZT[f, n_block] = sum over m chunks of x_chunk^T-form matmuls
                zt = psZ.tile([P, P], F32, tag="z")
                for t in range(nb_cnt):
                    tp = psT.tile([P, P], F32, tag="tp")
                    nc.tensor.transpose(tp, e[:, t * P:(t + 1) * P], ident)
                    et = etp.tile([P, P], F32, tag="et")
                    nc.vector.tensor_copy(et, tp)
                    if nb == 0:
                        # r0[m chunk t] = R[0, t*128 : (t+1)*128]
                        nc.vector.tensor_copy(r0[:, t:t + 1], et[:, 0:1])
                    nc.tensor.matmul(zt, lhsT=xnat[:, t, :], rhs=et,
                                     start=(t == 0), stop=(t == nb_cnt - 1))

                # h1T = relu(w1^T Z^T + xT_block); store natural via transpose
                ztsb = ztp.tile([P, P], F32, tag="zt")
                nc.vector.tensor_copy(ztsb, zt)
                hp = psH.tile([P, P], F32, tag="hp")
                nc.tensor.matmul(hp, lhsT=w1_sb, rhs=ztsb, start=True,
                                 stop=True)
                h1t = h1tp.tile([P, P], F32, tag="h1t")
                nc.vector.tensor_add(h1t, hp, xT[:, ncol:ncol + P])
                nc.vector.tensor_relu(h1t, h1t)
                tp2 = psT.tile([P, P], F32, tag="tp")
                nc.tensor.transpose(tp2, h1t, ident)
                nc.vector.tensor_copy(h1nat[:, nb, :], tp2)

            # v[f'] = sum_m R[0, m] h1[m, f']
            vps = psZ.tile([P, 1], F32, tag="z")
            for t in range(nb_cnt):
                nc.tensor.matmul(vps, lhsT=h1nat[:, t, :], rhs=r0[:, t:t + 1],
                                 start=(t == 0), stop=(t == nb_cnt - 1))
            vsb = stat.tile([P, 1], F32, tag="v")
            nc.vector.tensor_copy(vsb, vps)
            # out = relu(v @ w2 + h1[0, :])
            o2 = psH.tile([1, P], F32, tag="hp")
            nc.tensor.matmul(o2, lhsT=vsb, rhs=w2_sb, start=True, stop=True)
            fin = stat.tile([1, P], F32, tag="fin")
            nc.vector.tensor_add(fin, o2, h1nat[0:1, 0, :])
            nc.vector.tensor_relu(fin, fin)
            nc.sync.dma_start(out_d[:], fin)

    # Bacc pass pipeline: splits multi-sem waits into event-semaphore chains
    # (this walrus build allows at most one sync wait per instruction) and
    # moves extra matmul waits onto ldweights.
    nc.compile()
    return nc


_CACHE = {}


def kernel(x, w1, w2, wr):
    x = np.ascontiguousarray(np.asarray(x), dtype=np.float32)
    w1 = np.ascontiguousarray(np.asarray(w1), dtype=np.float32)
    w2 = np.ascontiguousarray(np.asarray(w2), dtype=np.float32)
    wr = np.ascontiguousarray(np.asarray(wr), dtype=np.float32)
    b, n, d = x.shape
    if "nc" not in _CACHE:
        _CACHE["nc"] = build_kernel(n)
    nc = _CACHE["nc"]
    in_maps = [{"x": x[i], "wr": wr, "w1": w1, "w2": w2} for i in range(b)]
    res = run_bass_kernel_spmd(nc, in_maps, core_ids=list(range(b)))
    return np.stack([res.results[i]["out"][0] for i in range(b)])


if __name__ == "__main__":
    xs = np.random.randn(B, 2048, D).astype(np.float32)
    ws = [np.random.randn(D, D).astype(np.float32) for _ in range(3)]
    out = kernel(xs, ws[0], ws[1], ws[2])
    print(out.shape, out.dtype)


# revision 8
# speedup vs baseline: 6.1625x; 6.1625x over previous
"""Instruction-minimal Trainium2 Bass kernel for nn_GCN_15590731285230 (v2.2).

On this rig every engine instruction costs ~25-45us of dispatch (globally
serialized), so the objective is MINIMUM TOTAL INSTRUCTION COUNT, not
engine-seconds. Design:

  * Scores only in transposed layout ST[m, n] = S[n, m] via fp32 matmuls
    (fp32 matmuls self-load weights: no separate Ldweights instruction).
  * Pass A fills one 8-bank PSUM tensor [128, 4096] (two m-chunks) per group,
    halving the PSUM->SBUF drain copies.
  * Softmax over m fused across the whole score tensor:
      - max: one strided DVE reduce over chunks + one gpsimd
        partition_all_reduce (arrives broadcast to all 128 partitions)
      - one tensor_sub over all 16 chunks using a stride-0 broadcast AP
      - one in-place Exp over the whole [128, 16*2048] tensor
      - sums: one strided DVE chunk-sum + one partition_all_reduce
      - normalization folded into Z: scale ZT columns by 1/s (exact: the same
        exp values feed the sums and the Z matmuls).
  * Layer-2 shortcut: only row 0 of layer 2 is needed;
    v = sum_n h1[n,:] R[0,n] via tensor_mul + reduce against a
    partition-broadcast of R's row 0 (extracted straight from the exp'd
    scores, column n=0).
  * SBUF slot reuse via pool tags: yT -> znorm -> wsum, mtile -> stile ->
    r0tile, mx_pt -> r0row (stall alone is 128KB/partition).

Per batch b (core b):
    R  = softmax(x wr x^T, axis=-1);  h1 = relu(R x w1 + x)
    out_b = relu(R[0,:] @ h1 @ w2 + h1[0,:])
"""

import sys

if "/opt/trn_rl_repo" not in sys.path:
    sys.path.insert(0, "/opt/trn_rl_repo")

from contextlib import ExitStack

import numpy as np

import concourse.bacc as bacc
import concourse.bass as bass
import concourse.bass_isa as bass_isa
import concourse.mybir as mybir
import concourse.tile as tile
from concourse.bass_utils import run_bass_kernel_spmd
from concourse.masks import make_identity

P = 128
D = 128
B = 8
F32 = mybir.dt.float32
AF = mybir.ActivationFunctionType
AX = mybir.AxisListType
ALU = mybir.AluOpType
ROP = bass_isa.ReduceOp


def _bcast_free(ap, count):
    """Insert a stride-0 dim of size `count` after the partition dim."""
    return bass.AP(tensor=ap.tensor, offset=ap.offset,
                   ap=[list(ap.ap[0]), [0, count]] + [list(d) for d in ap.ap[1:]])


def build_kernel(n=2048, repeat=1):
    nt = n // P              # m chunks
    w5 = min(512, n)         # matmul moving-operand width
    pair = 2 if nt % 2 == 0 else 1   # chunks per PSUM drain in pass A
    nc = bacc.Bacc()
    x_d = nc.dram_tensor("x", [n, D], F32, kind="ExternalInput")
    wr_d = nc.dram_tensor("wr", [D, D], F32, kind="ExternalInput")
    w1_d = nc.dram_tensor("w1", [D, D], F32, kind="ExternalInput")
    w2_d = nc.dram_tensor("w2", [D, D], F32, kind="ExternalInput")
    out_d = nc.dram_tensor("out", [1, D], F32, kind="ExternalOutput")

    with tile.TileContext(nc) as tc, ExitStack() as ctx:
        sg = ctx.enter_context(tc.tile_pool(name="sg", bufs=1))
        scr = ctx.enter_context(tc.tile_pool(name="scr", bufs=1))
        bb = ctx.enter_context(tc.tile_pool(name="bb", bufs=1))
        st = ctx.enter_context(tc.tile_pool(name="st", bufs=1))

        for _rep in range(repeat):
            ident = sg.tile([P, P], F32, tag="ident")
            make_identity(nc, ident)
            wr_sb = sg.tile([P, P], F32, tag="wr_sb")
            w1_sb = sg.tile([P, P], F32, tag="w1_sb")
            w2_sb = sg.tile([P, P], F32, tag="w2_sb")
            nc.sync.dma_start(wr_sb, wr_d[:])
            nc.sync.dma_start(w1_sb, w1_d[:])
            nc.sync.dma_start(w2_sb, w2_d[:])

            # One DMA: partition p holds x rows p*nt..p*nt+nt-1 (contiguous 8KB
            # per partition). This permutes the node enumeration to
            # g(p,t) = p*nt + t, which is consistent everywhere downstream
            # (scores, Z, h1 permute simultaneously; position 0 is still node
            # 0, and the output depends only on node 0's row).
            xnat = sg.tile([P, nt, P], F32, tag="xnat")
            nc.sync.dma_start(xnat, x_d[:].rearrange("(p t) f -> p t f", p=P))

            # xT via PE transposes, up to 4 per PSUM tile -> 1 copy per group
            xT = sg.tile([P, n], F32, tag="xT")
            yT = sg.tile([P, n], F32, tag="yT")
            with tc.tile_pool(name="pst", bufs=2, space="PSUM") as pst:
                gsz = min(4, nt)
                for g in range(0, nt, gsz):
                    tp = pst.tile([P, gsz * P], F32, tag="tp")
                    for k in range(gsz):
                        nc.tensor.transpose(tp[:, k * P:(k + 1) * P],
                                            xnat[:, g + k, :], ident)
                    nc.vector.tensor_copy(xT[:, g * P:(g + gsz) * P], tp)
                # yT = (x @ wr)^T : yT[g, n] = sum_f wr[f, g] xT[f, n]
                for j in range(0, n, w5):
                    yp = pst.tile([P, w5], F32, tag="tp")
                    nc.tensor.matmul(yp, lhsT=wr_sb, rhs=xT[:, j:j + w5],
                                     start=True, stop=True)
                    nc.vector.tensor_copy(yT[:, j:j + w5], yp)

            # ---- pass A: ST[m, n] = S[n, m], stored fp32 in SBUF ----
            stall = sg.tile([P, nt, n], F32, tag="stall")
            stall_flat = stall.rearrange("p t n -> p (t n)")
            with tc.tile_pool(name="psA", bufs=1, space="PSUM") as psA:
                for g in range(0, nt, pair):
                    sp = psA.tile([P, pair * n], F32, tag="sp")
                    for k in range(pair):
                        for j in range(0, n, w5):
                            nc.tensor.matmul(
                                sp[:, k * n + j:k * n + j + w5],
                                lhsT=xT[:, (g + k) * P:(g + k + 1) * P],
                                rhs=yT[:, j:j + w5],
                                start=True, stop=True)
                    nc.scalar.copy(
                        stall_flat[:, g * n:(g + pair) * n], sp)

            # ---- global column max over m (partitions x chunks) ----
            mx_pt = scr.tile([P, n], F32, tag="scr")
            nc.vector.tensor_reduce(mx_pt, stall.rearrange("p t n -> p n t"),
                                    axis=AX.X, op=ALU.max)
            mtile = bb.tile([P, n], F32, tag="bb")
            nc.gpsimd.partition_all_reduce(mtile, mx_pt, channels=P,
                                           reduce_op=ROP.max)

            # ---- softmax numerator: one sub + one in-place exp ----
            nc.vector.tensor_sub(stall_flat, stall_flat,
                                 _bcast_free(mtile[:], nt))
            nc.scalar.activation(stall_flat, stall_flat, AF.Exp)

            # column sums s[n] (over chunks, then partitions)
            etsum = scr.tile([P, n], F32, tag="scr")
            nc.vector.tensor_reduce(etsum, stall.rearrange("p t n -> p n t"),
                                    axis=AX.X, op=ALU.add)
            stile = bb.tile([P, n], F32, tag="bb")   # reuses mtile slot
            nc.gpsimd.partition_all_reduce(stile, etsum, channels=P,
                                           reduce_op=ROP.add)
            nc.vector.reciprocal(stile, stile)       # 1/s, broadcast

            # ---- Z^T accumulation over chunks (fp32, self-loading mms) ----
            with tc.tile_pool(name="psZ", bufs=1, space="PSUM") as psZ:
                ztp = psZ.tile([P, n], F32, tag="zt")
                for t in range(nt):
                    for j in range(0, n, w5):
                        nc.tensor.matmul(ztp[:, j:j + w5],
                                         lhsT=xnat[:, t, :],
                                         rhs=stall[:, t, j:j + w5],
                                         start=(t == 0), stop=(t == nt - 1))
                # znorm = ZT * (1/s): one op does PSUM->SBUF copy and scale
                znorm = sg.tile([P, n], F32, tag="yT")   # reuses yT slot
                nc.vector.tensor_mul(znorm, ztp, stile)

            # ---- h1T = relu(w1^T Znorm + xT) ----
            h1t = sg.tile([P, n], F32, tag="h1t")
            with tc.tile_pool(name="psH", bufs=1, space="PSUM") as psH:
                hp = psH.tile([P, n], F32, tag="hp")
                for j in range(0, n, w5):
                    nc.tensor.matmul(hp[:, j:j + w5], lhsT=w1_sb,
                                     rhs=znorm[:, j:j + w5],
                                     start=True, stop=True)
                nc.vector.tensor_add(h1t, hp, xT)
                nc.vector.tensor_relu(h1t, h1t)

            # ---- tail: out = relu(r0 @ h1 @ w2 + h1[0, :]) ----
            # r0 (unnormalized) = exp'd scores column n=0 = stall[:, :, 0]
            with tc.tile_pool(name="psT2", bufs=1, space="PSUM") as psT2:
                rtp = psT2.tile([nt, P], F32, tag="rtp")
                nc.tensor.transpose(
                    rtp, stall[:, :, 0:1].rearrange("p t o -> p (t o)"),
                    ident)
                r16 = st.tile([nt, P], F32, tag="r16")
                nc.vector.tensor_copy(r16, rtp)
                r0row = scr.tile([1, n], F32, tag="scr")
                nc.sync.dma_start(
                    r0row.rearrange("o (t p) -> o t p", t=nt), r16)
                # normalize by 1/s[0] (stile holds reciprocals, broadcast)
                nc.vector.tensor_scalar_mul(r0row, r0row, stile[0:1, 0:1])
                r0tile = bb.tile([P, n], F32, tag="bb")  # reuses stile slot
                nc.gpsimd.partition_broadcast(r0tile, r0row)
                wsum = sg.tile([P, n], F32, tag="yT")    # reuses znorm slot
                nc.vector.tensor_mul(wsum, h1t, r0tile)
                v = st.tile([P, 1], F32, tag="v")
                nc.vector.tensor_reduce(v, wsum, axis=AX.X, op=ALU.add)
                o2 = psT2.tile([1, P], F32, tag="o2")
                nc.tensor.matmul(o2, lhsT=v, rhs=w2_sb, start=True, stop=False)
                nc.tensor.matmul(o2, lhsT=h1t[:, 0:1], rhs=ident,
                                 start=False, stop=True)
                fin = st.tile([1, P], F32, tag="fin")
                nc.scalar.activation(fin, o2, AF.Relu)
                nc.sync.dma_start(out_d[:], fin)

    nc.compile()
    return nc


_CACHE = {}


def kernel(x, w1, w2, wr):
    x = np.ascontiguousarray(np.asarray(x), dtype=np.float32)
    w1 = np.ascontiguousarray(np.asarray(w1), dtype=np.float32)
    w2 = np.ascontiguousarray(np.asarray(w2), dtype=np.float32)
    wr = np.ascontiguousarray(np.asarray(wr), dtype=np.float32)
    b, n, d = x.shape
    if "nc" not in _CACHE:
        _CACHE["nc"] = build_kernel(n)
    nc = _CACHE["nc"]
    in_maps = [{"x": x[i], "wr": wr, "w1": w1, "w2": w2} for i in range(b)]
    res = run_bass_kernel_spmd(nc, in_maps, core_ids=list(range(b)))
    return np.stack([res.results[i]["out"][0] for i in range(b)])


# revision 11
# speedup vs baseline: 7.0994x; 1.1520x over previous
"""Instruction-minimal Trainium2 Bass kernel for nn_GCN_15590731285230 (v2.2).

On this rig every engine instruction costs ~25-45us of dispatch (globally
serialized), so the objective is MINIMUM TOTAL INSTRUCTION COUNT, not
engine-seconds. Design:

  * Scores only in transposed layout ST[m, n] = S[n, m] via fp32 matmuls
    (fp32 matmuls self-load weights: no separate Ldweights instruction).
  * Pass A fills one 8-bank PSUM tensor [128, 4096] (two m-chunks) per group,
    halving the PSUM->SBUF drain copies.
  * Softmax over m fused across the whole score tensor:
      - max: one strided DVE reduce over chunks + one gpsimd
        partition_all_reduce (arrives broadcast to all 128 partitions)
      - one tensor_sub over all 16 chunks using a stride-0 broadcast AP
      - one in-place Exp over the whole [128, 16*2048] tensor
      - sums: one strided DVE chunk-sum + one partition_all_reduce
      - normalization folded into Z: scale ZT columns by 1/s (exact: the same
        exp values feed the sums and the Z matmuls).
  * Layer-2 shortcut: only row 0 of layer 2 is needed;
    v = sum_n h1[n,:] R[0,n] via tensor_mul + reduce against a
    partition-broadcast of R's row 0 (extracted straight from the exp'd
    scores, column n=0).
  * SBUF slot reuse via pool tags: yT -> znorm -> wsum, mtile -> stile ->
    r0tile, mx_pt -> r0row (stall alone is 128KB/partition).

Per batch b (core b):
    R  = softmax(x wr x^T, axis=-1);  h1 = relu(R x w1 + x)
    out_b = relu(R[0,:] @ h1 @ w2 + h1[0,:])
"""

import sys

if "/opt/trn_rl_repo" not in sys.path:
    sys.path.insert(0, "/opt/trn_rl_repo")

from contextlib import ExitStack

import numpy as np

import concourse.bacc as bacc
import concourse.bass as bass
import concourse.bass_isa as bass_isa
import concourse.mybir as mybir
import concourse.tile as tile
from concourse.bass_utils import run_bass_kernel_spmd
from concourse.masks import make_identity

P = 128
D = 128
B = 8
F32 = mybir.dt.float32
AF = mybir.ActivationFunctionType
AX = mybir.AxisListType
ALU = mybir.AluOpType
ROP = bass_isa.ReduceOp


def _bcast_free(ap, count):
    """Insert a stride-0 dim of size `count` after the partition dim."""
    return bass.AP(tensor=ap.tensor, offset=ap.offset,
                   ap=[list(ap.ap[0]), [0, count]] + [list(d) for d in ap.ap[1:]])


def build_kernel(n=2048, repeat=1):
    nt = n // P              # m chunks
    w5 = min(512, n)         # matmul moving-operand width
    pair = 2 if nt % 2 == 0 else 1   # chunks per PSUM drain in pass A
    nc = bacc.Bacc()
    x_d = nc.dram_tensor("x", [n, D], F32, kind="ExternalInput")
    wr_d = nc.dram_tensor("wr", [D, D], F32, kind="ExternalInput")
    w1_d = nc.dram_tensor("w1", [D, D], F32, kind="ExternalInput")
    w2_d = nc.dram_tensor("w2", [D, D], F32, kind="ExternalInput")
    out_d = nc.dram_tensor("out", [1, D], F32, kind="ExternalOutput")

    with tile.TileContext(nc) as tc, ExitStack() as ctx:
        sg = ctx.enter_context(tc.tile_pool(name="sg", bufs=1))
        scr = ctx.enter_context(tc.tile_pool(name="scr", bufs=1))
        bb = ctx.enter_context(tc.tile_pool(name="bb", bufs=1))
        st = ctx.enter_context(tc.tile_pool(name="st", bufs=1))

        for _rep in range(repeat):
            ident = sg.tile([P, P], F32, tag="ident")
            make_identity(nc, ident)
            wr_sb = sg.tile([P, P], F32, tag="wr_sb")
            w1_sb = sg.tile([P, P], F32, tag="w1_sb")
            w2_sb = sg.tile([P, P], F32, tag="w2_sb")
            nc.sync.dma_start(wr_sb, wr_d[:])
            nc.sync.dma_start(w1_sb, w1_d[:])
            nc.sync.dma_start(w2_sb, w2_d[:])

            # One DMA: partition p holds x rows p*nt..p*nt+nt-1 (contiguous 8KB
            # per partition). This permutes the node enumeration to
            # g(p,t) = p*nt + t, which is consistent everywhere downstream
            # (scores, Z, h1 permute simultaneously; position 0 is still node
            # 0, and the output depends only on node 0's row).
            xnat = sg.tile([P, nt, P], F32, tag="xnat")
            nc.sync.dma_start(xnat, x_d[:].rearrange("(p t) f -> p t f", p=P))

            # xT via PE transposes packed into one wide PSUM tensor; then yT
            # matmuls into a second one. One drain copy per half instead of
            # one per 4-chunk group.
            xT = sg.tile([P, n], F32, tag="xT")
            yT = sg.tile([P, n], F32, tag="yT")
            with tc.tile_pool(name="pst", bufs=2, space="PSUM") as pst:
                half = max(n // 2, P)
                for h in range(0, n, half):
                    tp = pst.tile([P, half], F32, tag="tp")
                    for k in range(half // P):
                        nc.tensor.transpose(tp[:, k * P:(k + 1) * P],
                                            xnat[:, h // P + k, :], ident)
                    nc.vector.tensor_copy(xT[:, h:h + half], tp)
                # yT = (x @ wr)^T : yT[g, n] = sum_f wr[f, g] xT[f, n]
                wy = min(w5, half)
                for h in range(0, n, half):
                    yp = pst.tile([P, half], F32, tag="tp")
                    for j in range(0, half, wy):
                        nc.tensor.matmul(yp[:, j:j + wy], lhsT=wr_sb,
                                         rhs=xT[:, h + j:h + j + wy],
                                         start=True, stop=True)
                    nc.vector.tensor_copy(yT[:, h:h + half], yp)

            # ---- pass A: ST[m, n] = S[n, m], stored fp32 in SBUF ----
            stall = sg.tile([P, nt, n], F32, tag="stall")
            stall_flat = stall.rearrange("p t n -> p (t n)")
            with tc.tile_pool(name="psA", bufs=1, space="PSUM") as psA:
                for g in range(0, nt, pair):
                    sp = psA.tile([P, pair * n], F32, tag="sp")
                    for k in range(pair):
                        for j in range(0, n, w5):
                            nc.tensor.matmul(
                                sp[:, k * n + j:k * n + j + w5],
                                lhsT=xT[:, (g + k) * P:(g + k + 1) * P],
                                rhs=yT[:, j:j + w5],
                                start=True, stop=True)
                    nc.scalar.copy(
                        stall_flat[:, g * n:(g + pair) * n], sp)

            # ---- global column max over m (partitions x chunks) ----
            mx_pt = scr.tile([P, n], F32, tag="scr")
            nc.vector.tensor_reduce(mx_pt, stall.rearrange("p t n -> p n t"),
                                    axis=AX.X, op=ALU.max)
            mtile = bb.tile([P, n], F32, tag="bb")
            nc.gpsimd.partition_all_reduce(mtile, mx_pt, channels=P,
                                           reduce_op=ROP.max)

            # ---- softmax numerator: one sub + one in-place exp ----
            nc.vector.tensor_sub(stall_flat, stall_flat,
                                 _bcast_free(mtile[:], nt))
            nc.scalar.activation(stall_flat, stall_flat, AF.Exp)

            # column sums s[n] (over chunks, then partitions)
            etsum = scr.tile([P, n], F32, tag="scr")
            nc.vector.tensor_reduce(etsum, stall.rearrange("p t n -> p n t"),
                                    axis=AX.X, op=ALU.add)
            stile = bb.tile([P, n], F32, tag="bb")   # reuses mtile slot
            nc.gpsimd.partition_all_reduce(stile, etsum, channels=P,
                                           reduce_op=ROP.add)
            nc.vector.reciprocal(stile, stile)       # 1/s, broadcast

            # ---- Z^T accumulation over chunks (fp32, self-loading mms) ----
            with tc.tile_pool(name="psB", bufs=1, space="PSUM") as psB:
                ztp = psB.tile([P, n], F32, tag="zt")
                for t in range(nt):
                    for j in range(0, n, w5):
                        nc.tensor.matmul(ztp[:, j:j + w5],
                                         lhsT=xnat[:, t, :],
                                         rhs=stall[:, t, j:j + w5],
                                         start=(t == 0), stop=(t == nt - 1))
                # znorm = ZT * (1/s): one op does PSUM->SBUF copy and scale
                znorm = sg.tile([P, n], F32, tag="yT")   # reuses yT slot
                nc.vector.tensor_mul(znorm, ztp, stile)

                # ---- h1T = relu(w1^T Znorm + xT) ----
                h1t = sg.tile([P, n], F32, tag="h1t")
                hp = psB.tile([P, n], F32, tag="hp")
                for j in range(0, n, w5):
                    nc.tensor.matmul(hp[:, j:j + w5], lhsT=w1_sb,
                                     rhs=znorm[:, j:j + w5],
                                     start=True, stop=True)
                nc.vector.tensor_add(h1t, hp, xT)
                nc.vector.tensor_relu(h1t, h1t)

                # ---- tail: out = relu(r0 @ h1 @ w2 + h1[0, :]) ----
                # r0 (unnormalized) = exp'd scores column n=0 = stall[:, :, 0]
                rtp = psB.tile([nt, P], F32, tag="zt")  # reuses ztp banks
                nc.tensor.transpose(
                    rtp, stall[:, :, 0:1].rearrange("p t o -> p (t o)"),
                    ident)
                r16 = st.tile([nt, P], F32, tag="r16")
                nc.vector.tensor_copy(r16, rtp)
                r0row = scr.tile([1, n], F32, tag="scr")
                nc.sync.dma_start(
                    r0row.rearrange("o (t p) -> o t p", t=nt), r16)
                # normalize by 1/s[0] (stile holds reciprocals, broadcast)
                nc.vector.tensor_scalar_mul(r0row, r0row, stile[0:1, 0:1])
                r0tile = bb.tile([P, n], F32, tag="bb")  # reuses stile slot
                nc.gpsimd.partition_broadcast(r0tile, r0row)
                wsum = sg.tile([P, n], F32, tag="yT")    # reuses znorm slot
                nc.vector.tensor_mul(wsum, h1t, r0tile)
                v = st.tile([P, 1], F32, tag="v")
                nc.vector.tensor_reduce(v, wsum, axis=AX.X, op=ALU.add)
                o2 = psB.tile([1, P], F32, tag="hp")  # reuses hp banks
                nc.tensor.matmul(o2, lhsT=v, rhs=w2_sb, start=True, stop=False)
                nc.tensor.matmul(o2, lhsT=h1t[:, 0:1], rhs=ident,
                                 start=False, stop=True)
                fin = st.tile([1, P], F32, tag="fin")
                nc.scalar.activation(fin, o2, AF.Relu)
                nc.sync.dma_start(out_d[:], fin)

    nc.compile()
    return nc


_CACHE = {}


def kernel(x, w1, w2, wr):
    x = np.ascontiguousarray(np.asarray(x), dtype=np.float32)
    w1 = np.ascontiguousarray(np.asarray(w1), dtype=np.float32)
    w2 = np.ascontiguousarray(np.asarray(w2), dtype=np.float32)
    wr = np.ascontiguousarray(np.asarray(wr), dtype=np.float32)
    b, n, d = x.shape
    if "nc" not in _CACHE:
        _CACHE["nc"] = build_kernel(n)
    nc = _CACHE["nc"]
    in_maps = [{"x": x[i], "wr": wr, "w1": w1, "w2": w2} for i in range(b)]
    res = run_bass_kernel_spmd(nc, in_maps, core_ids=list(range(b)))
    return np.stack([res.results[i]["out"][0] for i in range(b)])
